# revision 18
# baseline (speedup 1.0000x reference)
"""KoLeo loss kernel for Trainium2, 8 NeuronCores (SPMD), fp8 gram.

reference math:
    x = thought_vectors.reshape(-1, D)          # [N, D], N=8192, D=1024
    xn = x / max(||x||, 1e-12)
    sim = min(xn @ xn.T, 1.0)
    dist = sqrt(2 - 2*sim + 1e-4), diag -> inf
    loss = -mean(log(min_row_dist + 1e-8))

Key reduction: dist is monotone-decreasing in sim, so
    min_dist_i = sqrt(2 - 2*min(max_{j!=i} sim_ij, 1) + 1e-4)
and we only need a row-max of the Gram matrix (diag excluded).

Sharding: rows of x across 8 cores (1024 rows each). Each core
normalizes its shard scaled by 16, transposes it to [D, n] and stores
fp8e4 (values ~N(0, 0.25); quantization error ~2e-3 on the loss, far
inside the 2e-2 gate). The transposed fp8 shards are AllGathered
(1MB/rank), and each core computes its [1024, 8192] block of the
256x-scaled Gram with DoubleRow fp8 matmuls (2x TensorE throughput).

Diagonal exclusion runs on the TensorE too: an extra DoubleRow matmul
accumulates (16I)^T @ dfix[j] into the diagonal PSUM sub-block, where
dfix[j] (per-core input data) is -16I exactly for the two chunks that
contain this core's diagonal and 0 elsewhere, i.e. it subtracts an
exact 256 from the diagonal (residual |sim_ii*256 - 256| < 3 vs
row-max >= 29, so the diagonal never wins the max).

Row-max: matmuls for 4 m-tiles accumulate into one 4-bank PSUM tile
[128, 4, 512]; a single DVE reduce_max covers all 4 (fewer DVE
instructions, bigger ops), ping-ponging with the other 4 banks.
Final clamp/sqrt/log on-chip; host sums the 8x[128x8] outputs.
"""

import numpy as np

_P = 128
_EPS_NORM = 1e-12
_EPS_DIST = 1e-4
_EPSILON = 1e-8
_SCALE = 16.0  # fp8 pre-scale; gram is _SCALE**2 * sim


def _build_program(ncores, NB, D, CHUNK):
    """Build the SPMD Bass program (one program, runs on all cores).

    NB: rows per core. D: feature dim. CHUNK: gram columns per chunk.
    """
    import concourse.bacc as bacc
    import concourse.mybir as mybir
    from concourse.tile import TileContext
    from concourse.masks import make_identity

    f32 = mybir.dt.float32
    bf16 = mybir.dt.bfloat16
    fp8 = mybir.dt.float8e4
    P = _P
    M_TILES = NB // P
    K_TILES = D // P
    K2 = K_TILES // 2  # DoubleRow k-pair count
    N = NB * ncores
    NCHUNK = N // CHUNK
    CP = CHUNK // P  # m-tiles per chunk width
    MG = 2  # m-tiles per PSUM group (2 banks)
    S2 = _SCALE * _SCALE
    assert NB % CHUNK == 0 and CHUNK % P == 0

    nc = bacc.Bacc(
        "TRN2", target_bir_lowering=False, debug=False, num_devices=ncores
    )
    xs = nc.dram_tensor("xs", [NB, D], f32, kind="ExternalInput")

    out = nc.dram_tensor("out", [P, M_TILES], f32, kind="ExternalOutput")

    with TileContext(nc) as tc:
        with (
            tc.tile_pool(name="consts", bufs=1) as consts,
            tc.tile_pool(name="dram", bufs=1, space="DRAM") as dram,
            tc.tile_pool(name="small", bufs=4) as small,
        ):
            identity = consts.tile([P, P], bf16)
            make_identity(nc, identity)
            pos16 = consts.tile([P, 2, P], fp8)
            nc.vector.memset(pos16, 0.0)
            nc.vector.tensor_scalar_mul(pos16[:, 0, :], identity, _SCALE)
            neg16 = consts.tile([P, 2, P], fp8)
            nc.vector.memset(neg16, 0.0)
            nc.vector.tensor_scalar_mul(neg16[:, 0, :], identity, -_SCALE)
            bias_dist = consts.tile([P, 1], f32)
            nc.vector.memset(bias_dist, 2.0 + _EPS_DIST)
            bias_log = consts.tile([P, 1], f32)
            nc.vector.memset(bias_log, _EPSILON)
            # resident transposed normalized shard (fp8, x16), split in
            # halves so the first AllGather's staging only depends on the
            # first half of the pre-pass (Tile deps are tile-granular)
            MH = M_TILES // 2
            xnT_A = consts.tile([P, K_TILES, MH * P], fp8)
            xnT_B = consts.tile([P, K_TILES, MH * P], fp8)
            maxacc = consts.tile([P, M_TILES, NCHUNK], f32)
            outt = consts.tile([P, M_TILES], f32)

            HB = NB // 2
            xnT_localA = dram.tile([D, HB], fp8)
            xnT_localB = dram.tile([D, HB], fp8)
            xnT_allA = dram.tile([ncores * D, HB], fp8, addr_space="Shared")
            xnT_allB = dram.tile([ncores * D, HB], fp8, addr_space="Shared")

            # ---- pre-pass + own-block chunks, interleaved so the PE's
            # own-chunk burst is split around the second prepass half (a
            # concentrated burst trips the 50%-cap activity throttle over
            # the following AllGathers) ----
            from concourse.bass import ds

            with (
                tc.tile_pool(name="prep", bufs=3) as prep,
                tc.tile_pool(name="ppsum", bufs=2, space="PSUM") as ppsum,
                tc.tile_pool(name="rhsp", bufs=14) as rhsp,
                tc.tile_pool(name="mpsum", bufs=3, space="PSUM") as mpsum,
            ):
                def gram_groups(lhsAB, rhs_fn, slot, fix_h, groups):
                    """Emit PSUM groups `groups` (of M_TILES//MG) for one
                    CHUNK-wide column chunk into maxacc slot."""
                    for mg in groups:
                        psg = mpsum.tile([P, MG, CHUNK], f32, tag="ps")
                        for mi in range(MG):
                            m = mg * MG + mi
                            ps = psg[:, mi, :]
                            lhs_h = lhsAB[0] if m < MH else lhsAB[1]
                            lmc = (m % MH) * P
                            has_fix = fix_h is not None and m // CP == fix_h
                            for k2 in range(K2):
                                nc.tensor.matmul(
                                    ps,
                                    lhs_h[:, 2 * k2 : 2 * k2 + 2, lmc : lmc + P],
                                    rhs_fn(k2),
                                    start=(k2 == 0),
                                    stop=(k2 == K2 - 1) and not has_fix,
                                    perf_mode=mybir.MatmulPerfMode.DoubleRow,
                                )
                            if has_fix:
                                off = (m % CP) * P
                                nc.tensor.matmul(
                                    ps[:, off : off + P],
                                    pos16,
                                    neg16,
                                    start=False,
                                    stop=True,
                                    perf_mode=mybir.MatmulPerfMode.DoubleRow,
                                    skip_group_check=True,
                                )
                        nc.vector.reduce_max(
                            maxacc[:, mg * MG : (mg + 1) * MG, slot : slot + 1],
                            psg,
                            axis=mybir.AxisListType.X,
                        )

                def rhsA(k2):
                    return xnT_A[:, 2 * k2 : 2 * k2 + 2, :]

                def rhsB(k2):
                    return xnT_B[:, 2 * k2 : 2 * k2 + 2, :]

                NGRP = M_TILES // MG
                for m in range(M_TILES):
                    xt = prep.tile([P, D], f32, tag="xt")
                    nc.sync.dma_start(xt, xs[m * P : (m + 1) * P, :])
                    sq = prep.tile([P, D], bf16, tag="sq")
                    ss = small.tile([P, 1], f32, tag="ss")
                    nc.scalar.activation(
                        sq,
                        xt,
                        mybir.ActivationFunctionType.Square,
                        accum_out=ss,
                    )
                    nrm = small.tile([P, 1], f32, tag="nrm")
                    nc.scalar.sqrt(nrm, ss)
                    nrm2 = small.tile([P, 1], f32, tag="nrm2")
                    nc.vector.tensor_scalar_max(nrm2, nrm, _EPS_NORM)
                    rinv = small.tile([P, 1], f32, tag="rinv")
                    nc.vector.reciprocal(rinv, nrm2)
                    rinv16 = small.tile([P, 1], f32, tag="rinv16")
                    nc.vector.tensor_scalar_mul(rinv16, rinv, _SCALE)
                    xnb = prep.tile([P, D], bf16, tag="xnb")
                    nc.scalar.mul(xnb, xt, rinv16)
                    for k in range(K_TILES):
                        pt = ppsum.tile([P, P], bf16, tag="pt")
                        nc.tensor.transpose(
                            pt, xnb[:, k * P : (k + 1) * P], identity
                        )
                        xnT_h = xnT_A if m < MH else xnT_B
                        nc.vector.tensor_copy(
                            xnT_h[:, k, (m % MH) * P : (m % MH + 1) * P], pt
                        )
                    if m == M_TILES // 2 - 1:
                        nc.sync.dma_start(
                            xnT_localA[:, :].rearrange(
                                "(k p) n -> p k n", p=P
                            ),
                            xnT_A,
                        )
                        nc.gpsimd.collective_compute(
                            "AllGather",
                            mybir.AluOpType.bypass,
                            replica_groups=[list(range(ncores))],
                            ins=[xnT_localA.opt()],
                            outs=[xnT_allA.opt()],
                        )
                        # A-only own-chunk groups fill the gap while the
                        # second prepass half runs (splits the PE burst)
                        gram_groups(
                            (xnT_A, xnT_B), rhsA, 0, 0, range(NGRP // 2)
                        )
                    elif m == M_TILES - 1:
                        nc.sync.dma_start(
                            xnT_localB[:, :].rearrange(
                                "(k p) n -> p k n", p=P
                            ),
                            xnT_B,
                        )
                        nc.gpsimd.collective_compute(
                            "AllGather",
                            mybir.AluOpType.bypass,
                            replica_groups=[list(range(ncores))],
                            ins=[xnT_localB.opt()],
                            outs=[xnT_allB.opt()],
                        )
                # remaining own-chunk groups
                gram_groups(
                    (xnT_A, xnT_B), rhsA, 0, 0, range(NGRP // 2, NGRP)
                )
                gram_groups((xnT_A, xnT_B), rhsB, 1, 1, range(NGRP))

                # remote blocks (slots 2..15): rank-steered source offset
                rank = nc.sync.cc_rank(replica_groups=[list(range(ncores))])
                for half in range(2):
                    src = xnT_allA if half == 0 else xnT_allB
                    for i in range(ncores - 1):
                        blk = (rank + (1 + i)) & 7
                        rt = rhsp.tile([P, K_TILES, CHUNK], fp8, tag="rhs")
                        nc.sync.dma_start(
                            rt,
                            src[ds(blk * D, D), :].rearrange(
                                "(k p) c -> p k c", p=P
                            ),
                        )
                        slot = 2 + half * (ncores - 1) + i
                        gram_groups(
                            (xnT_A, xnT_B),
                            lambda k2, r=rt: r[:, 2 * k2 : 2 * k2 + 2, :],
                            slot,
                            None,
                            range(NGRP),
                        )

            # ---- final: clamp, dist, log (vectorized over all m) ----
            mx = small.tile([P, M_TILES], f32, tag="mx")
            nc.vector.reduce_max(mx, maxacc, axis=mybir.AxisListType.X)
            mxc = small.tile([P, M_TILES], f32, tag="mxc")
            nc.vector.tensor_scalar_min(mxc, mx, S2)
            dst = small.tile([P, M_TILES], f32, tag="dst")
            nc.scalar.activation(
                dst,
                mxc,
                mybir.ActivationFunctionType.Sqrt,
                bias=bias_dist,
                scale=-2.0 / S2,
            )
            nc.scalar.activation(
                outt,
                dst,
                mybir.ActivationFunctionType.Ln,
                bias=bias_log,
                scale=1.0,
            )
            nc.sync.dma_start(out[:, :], outt)

    nc.compile()
    return nc


def _run(thought_vectors, trace=False, tmpdir=None):
    from concourse import mybir
    from concourse.bass_utils import run_bass_kernel_spmd


    ncores, NB, D, CHUNK = 8, 1024, 1024, 512
    x = np.ascontiguousarray(
        np.asarray(thought_vectors, dtype=np.float32).reshape(-1, D)
    )
    N = x.shape[0]
    assert N == ncores * NB

    nc = _build_program(ncores, NB, D, CHUNK)

    in_maps = []
    for c in range(ncores):
        in_maps.append({"xs": x[c * NB : (c + 1) * NB]})

    res = run_bass_kernel_spmd(
        nc,
        in_maps,
        core_ids=list(range(ncores)),
        trace=trace,
        tmpdir=tmpdir,
    )

    total = 0.0
    for c in range(ncores):
        total += float(np.asarray(res.results[c]["out"], dtype=np.float64).sum())
    loss = -total / N
    return np.float32(loss), res


def kernel(thought_vectors):
    loss, _ = _run(thought_vectors)
    return np.asarray(loss, dtype=np.float32)


# revision 19
# speedup vs baseline: 1.2018x; 1.2018x over previous
"""KoLeo loss kernel for Trainium2, 8 NeuronCores (SPMD), fp8 gram.

reference math:
    x = thought_vectors.reshape(-1, D)          # [N, D], N=8192, D=1024
    xn = x / max(||x||, 1e-12)
    sim = min(xn @ xn.T, 1.0)
    dist = sqrt(2 - 2*sim + 1e-4), diag -> inf
    loss = -mean(log(min_row_dist + 1e-8))

Key reduction: dist is monotone-decreasing in sim, so
    min_dist_i = sqrt(2 - 2*min(max_{j!=i} sim_ij, 1) + 1e-4)
and we only need a row-max of the Gram matrix (diag excluded).

Sharding: rows of x across 8 cores (1024 rows each). Each core
normalizes its shard scaled by 16, transposes it to [D, n] and stores
fp8e4 (values ~N(0, 0.25); quantization error ~2e-3 on the loss, far
inside the 2e-2 gate). The transposed fp8 shards are AllGathered
(1MB/rank), and each core computes its [1024, 8192] block of the
256x-scaled Gram with DoubleRow fp8 matmuls (2x TensorE throughput).

Diagonal exclusion runs on the TensorE too: an extra DoubleRow matmul
accumulates (16I)^T @ dfix[j] into the diagonal PSUM sub-block, where
dfix[j] (per-core input data) is -16I exactly for the two chunks that
contain this core's diagonal and 0 elsewhere, i.e. it subtracts an
exact 256 from the diagonal (residual |sim_ii*256 - 256| < 3 vs
row-max >= 29, so the diagonal never wins the max).

Row-max: matmuls for 4 m-tiles accumulate into one 4-bank PSUM tile
[128, 4, 512]; a single DVE reduce_max covers all 4 (fewer DVE
instructions, bigger ops), ping-ponging with the other 4 banks.
Final clamp/sqrt/log on-chip; host sums the 8x[128x8] outputs.
"""

import numpy as np

_P = 128
_EPS_NORM = 1e-12
_EPS_DIST = 1e-4
_EPSILON = 1e-8
_SCALE = 16.0  # fp8 pre-scale; gram is _SCALE**2 * sim


def _build_program(ncores, NB, D, CHUNK):
    """Build the SPMD Bass program (one program, runs on all cores).

    NB: rows per core. D: feature dim. CHUNK: gram columns per chunk.
    """
    import concourse.bacc as bacc
    import concourse.mybir as mybir
    from concourse.tile import TileContext
    from concourse.masks import make_identity

    f32 = mybir.dt.float32
    bf16 = mybir.dt.bfloat16
    fp8 = mybir.dt.float8e4
    P = _P
    M_TILES = NB // P
    K_TILES = D // P
    K2 = K_TILES // 2  # DoubleRow k-pair count
    N = NB * ncores
    NCHUNK = N // CHUNK
    CP = CHUNK // P  # m-tiles per chunk width
    MG = 4  # m-tiles per PSUM group (4 banks)
    S2 = _SCALE * _SCALE
    assert NB % CHUNK == 0 and CHUNK % P == 0

    nc = bacc.Bacc(
        "TRN2", target_bir_lowering=False, debug=False, num_devices=ncores
    )
    xs = nc.dram_tensor("xs", [NB, D], f32, kind="ExternalInput")

    out = nc.dram_tensor("out", [P, M_TILES], f32, kind="ExternalOutput")

    with TileContext(nc) as tc:
        with (
            tc.tile_pool(name="consts", bufs=1) as consts,
            tc.tile_pool(name="dram", bufs=1, space="DRAM") as dram,
            tc.tile_pool(name="small", bufs=4) as small,
        ):
            identity = consts.tile([P, P], bf16)
            make_identity(nc, identity)
            pos16 = consts.tile([P, 2, P], fp8)
            nc.vector.memset(pos16, 0.0)
            nc.vector.tensor_scalar_mul(pos16[:, 0, :], identity, _SCALE)
            neg16 = consts.tile([P, 2, P], fp8)
            nc.vector.memset(neg16, 0.0)
            nc.vector.tensor_scalar_mul(neg16[:, 0, :], identity, -_SCALE)
            bias_dist = consts.tile([P, 1], f32)
            nc.vector.memset(bias_dist, 2.0 + _EPS_DIST)
            bias_log = consts.tile([P, 1], f32)
            nc.vector.memset(bias_log, _EPSILON)
            # resident transposed normalized shard (fp8, x16), split in
            # halves so the first AllGather's staging only depends on the
            # first half of the pre-pass (Tile deps are tile-granular)
            MH = M_TILES // 2
            xnT_A = consts.tile([P, K_TILES, MH * P], fp8)
            xnT_B = consts.tile([P, K_TILES, MH * P], fp8)
            maxacc = consts.tile([P, M_TILES, NCHUNK], f32)
            outt = consts.tile([P, M_TILES], f32)

            HB = NB // 2
            xnT_localA = dram.tile([D, HB], fp8)
            xnT_localB = dram.tile([D, HB], fp8)
            xnT_allA = dram.tile([ncores * D, HB], fp8, addr_space="Shared")
            xnT_allB = dram.tile([ncores * D, HB], fp8, addr_space="Shared")

            # ---- pre-pass: normalize own shard, transpose, stage for AG ----
            with (
                tc.tile_pool(name="prep", bufs=3) as prep,
                tc.tile_pool(name="ppsum", bufs=4, space="PSUM") as ppsum,
            ):
                for m in range(M_TILES):
                    xt = prep.tile([P, D], f32, tag="xt")
                    nc.sync.dma_start(xt, xs[m * P : (m + 1) * P, :])
                    sq = prep.tile([P, D], bf16, tag="sq")
                    ss = small.tile([P, 1], f32, tag="ss")
                    nc.scalar.activation(
                        sq,
                        xt,
                        mybir.ActivationFunctionType.Square,
                        accum_out=ss,
                    )
                    nrm = small.tile([P, 1], f32, tag="nrm")
                    nc.scalar.sqrt(nrm, ss)
                    nrm2 = small.tile([P, 1], f32, tag="nrm2")
                    nc.vector.tensor_scalar_max(nrm2, nrm, _EPS_NORM)
                    rinv = small.tile([P, 1], f32, tag="rinv")
                    nc.vector.reciprocal(rinv, nrm2)
                    rinv16 = small.tile([P, 1], f32, tag="rinv16")
                    nc.vector.tensor_scalar_mul(rinv16, rinv, _SCALE)
                    xnb = prep.tile([P, D], bf16, tag="xnb")
                    nc.scalar.mul(xnb, xt, rinv16)
                    for k in range(K_TILES):
                        pt = ppsum.tile([P, P], bf16, tag="pt")
                        nc.tensor.transpose(
                            pt, xnb[:, k * P : (k + 1) * P], identity
                        )
                        xnT_h = xnT_A if m < MH else xnT_B
                        nc.vector.tensor_copy(
                            xnT_h[:, k, (m % MH) * P : (m % MH + 1) * P], pt
                        )
                    # stage + gather each half as soon as its m-tiles are
                    # done so the first AllGather overlaps the second half of
                    # the pre-pass (and the launch-skew barrier).
                    if m == M_TILES // 2 - 1:
                        nc.sync.dma_start(
                            xnT_localA[:, :].rearrange(
                                "(k p) n -> p k n", p=P
                            ),
                            xnT_A,
                        )
                        nc.gpsimd.collective_compute(
                            "AllGather",
                            mybir.AluOpType.bypass,
                            replica_groups=[list(range(ncores))],
                            ins=[xnT_localA.opt()],
                            outs=[xnT_allA.opt()],
                        )
                    elif m == M_TILES - 1:
                        nc.sync.dma_start(
                            xnT_localB[:, :].rearrange(
                                "(k p) n -> p k n", p=P
                            ),
                            xnT_B,
                        )
                        nc.gpsimd.collective_compute(
                            "AllGather",
                            mybir.AluOpType.bypass,
                            replica_groups=[list(range(ncores))],
                            ins=[xnT_localB.opt()],
                            outs=[xnT_allB.opt()],
                        )

            # ---- main pass ----
            # Own-block chunks run straight out of SBUF with a compile-time
            # diagonal fix, overlapping the collective launch barrier; the
            # 14 remote chunks are fetched with a cc_rank-steered dynamic
            # DMA offset ((rank+1+i) & 7 source block), so the throttled
            # main loop does 14/16 of the work.
            from concourse.bass import ds

            with (
                tc.tile_pool(name="rhsp", bufs=14) as rhsp,
                tc.tile_pool(name="mpsum", bufs=2, space="PSUM") as mpsum,
            ):
                def gram_chunk(lhsAB, rhs_fn, slot, fix_h):
                    """One CHUNK-wide column chunk; rhs_fn(k2) -> [P,2,CHUNK]
                    moving AP. fix_h: chunk-half for the static diag fix or
                    None."""
                    for mg in range(M_TILES // MG):
                        psg = mpsum.tile([P, MG, CHUNK], f32, tag="ps")
                        for mi in range(MG):
                            m = mg * MG + mi
                            ps = psg[:, mi, :]
                            lhs_h = lhsAB[0] if m < MH else lhsAB[1]
                            lmc = (m % MH) * P
                            has_fix = fix_h is not None and m // CP == fix_h
                            for k2 in range(K2):
                                nc.tensor.matmul(
                                    ps,
                                    lhs_h[:, 2 * k2 : 2 * k2 + 2, lmc : lmc + P],
                                    rhs_fn(k2),
                                    start=(k2 == 0),
                                    stop=(k2 == K2 - 1) and not has_fix,
                                    perf_mode=mybir.MatmulPerfMode.DoubleRow,
                                )
                            if has_fix:
                                off = (m % CP) * P
                                nc.tensor.matmul(
                                    ps[:, off : off + P],
                                    pos16,
                                    neg16,
                                    start=False,
                                    stop=True,
                                    perf_mode=mybir.MatmulPerfMode.DoubleRow,
                                    skip_group_check=True,
                                )
                        nc.vector.reduce_max(
                            maxacc[:, mg * MG : (mg + 1) * MG, slot : slot + 1],
                            psg,
                            axis=mybir.AxisListType.X,
                        )

                # own block (slots 0,1): no AllGather dependency
                for h in range(2):
                    src_own = xnT_A if h == 0 else xnT_B
                    gram_chunk(
                        (xnT_A, xnT_B),
                        lambda k2, s=src_own: s[:, 2 * k2 : 2 * k2 + 2, :],
                        h,
                        h,
                    )

                # remote blocks (slots 2..15): rank-steered source offset
                rank = nc.sync.cc_rank(replica_groups=[list(range(ncores))])
                for half in range(2):
                    src = xnT_allA if half == 0 else xnT_allB
                    for i in range(ncores - 1):
                        blk = (rank + (1 + i)) & 7
                        rt = rhsp.tile([P, K_TILES, CHUNK], fp8, tag="rhs")
                        nc.sync.dma_start(
                            rt,
                            src[ds(blk * D, D), :].rearrange(
                                "(k p) c -> p k c", p=P
                            ),
                        )
                        slot = 2 + half * (ncores - 1) + i
                        gram_chunk(
                            (xnT_A, xnT_B),
                            lambda k2, r=rt: r[:, 2 * k2 : 2 * k2 + 2, :],
                            slot,
                            None,
                        )

            # ---- final: clamp, dist, log (vectorized over all m) ----
            mx = small.tile([P, M_TILES], f32, tag="mx")
            nc.vector.reduce_max(mx, maxacc, axis=mybir.AxisListType.X)
            mxc = small.tile([P, M_TILES], f32, tag="mxc")
            nc.vector.tensor_scalar_min(mxc, mx, S2)
            dst = small.tile([P, M_TILES], f32, tag="dst")
            nc.scalar.activation(
                dst,
                mxc,
                mybir.ActivationFunctionType.Sqrt,
                bias=bias_dist,
                scale=-2.0 / S2,
            )
            nc.scalar.activation(
                outt,
                dst,
                mybir.ActivationFunctionType.Ln,
                bias=bias_log,
                scale=1.0,
            )
            nc.sync.dma_start(out[:, :], outt)

    nc.compile()
    return nc


def _run(thought_vectors, trace=False, tmpdir=None):
    from concourse import mybir
    from concourse.bass_utils import run_bass_kernel_spmd


    ncores, NB, D, CHUNK = 8, 1024, 1024, 512
    x = np.ascontiguousarray(
        np.asarray(thought_vectors, dtype=np.float32).reshape(-1, D)
    )
    N = x.shape[0]
    assert N == ncores * NB

    nc = _build_program(ncores, NB, D, CHUNK)

    in_maps = []
    for c in range(ncores):
        in_maps.append({"xs": x[c * NB : (c + 1) * NB]})

    res = run_bass_kernel_spmd(
        nc,
        in_maps,
        core_ids=list(range(ncores)),
        trace=trace,
        tmpdir=tmpdir,
    )

    total = 0.0
    for c in range(ncores):
        total += float(np.asarray(res.results[c]["out"], dtype=np.float64).sum())
    loss = -total / N
    return np.float32(loss), res


def kernel(thought_vectors):
    loss, _ = _run(thought_vectors)
    return np.asarray(loss, dtype=np.float32)


# revision 20
# speedup vs baseline: 1.2605x; 1.0489x over previous
"""KoLeo loss kernel for Trainium2, 8 NeuronCores (SPMD), fp8 gram.

reference math:
    x = thought_vectors.reshape(-1, D)          # [N, D], N=8192, D=1024
    xn = x / max(||x||, 1e-12)
    sim = min(xn @ xn.T, 1.0)
    dist = sqrt(2 - 2*sim + 1e-4), diag -> inf
    loss = -mean(log(min_row_dist + 1e-8))

Key reduction: dist is monotone-decreasing in sim, so
    min_dist_i = sqrt(2 - 2*min(max_{j!=i} sim_ij, 1) + 1e-4)
and we only need a row-max of the Gram matrix (diag excluded).

Sharding: rows of x across 8 cores (1024 rows each). Each core
normalizes its shard scaled by 16, transposes it to [D, n] and stores
fp8e4 (values ~N(0, 0.25); quantization error ~2e-3 on the loss, far
inside the 2e-2 gate). The transposed fp8 shards are AllGathered
(1MB/rank), and each core computes its [1024, 8192] block of the
256x-scaled Gram with DoubleRow fp8 matmuls (2x TensorE throughput).

Diagonal exclusion runs on the TensorE too: an extra DoubleRow matmul
accumulates (16I)^T @ dfix[j] into the diagonal PSUM sub-block, where
dfix[j] (per-core input data) is -16I exactly for the two chunks that
contain this core's diagonal and 0 elsewhere, i.e. it subtracts an
exact 256 from the diagonal (residual |sim_ii*256 - 256| < 3 vs
row-max >= 29, so the diagonal never wins the max).

Row-max: matmuls for 4 m-tiles accumulate into one 4-bank PSUM tile
[128, 4, 512]; a single DVE reduce_max covers all 4 (fewer DVE
instructions, bigger ops), ping-ponging with the other 4 banks.
Final clamp/sqrt/log on-chip; host sums the 8x[128x8] outputs.
"""

import numpy as np

_P = 128
_EPS_NORM = 1e-12
_EPS_DIST = 1e-4
_EPSILON = 1e-8
_SCALE = 16.0  # fp8 pre-scale; gram is _SCALE**2 * sim


def _build_program(ncores, NB, D, CHUNK):
    """Build the SPMD Bass program (one program, runs on all cores).

    NB: rows per core. D: feature dim. CHUNK: gram columns per chunk.
    """
    import concourse.bacc as bacc
    import concourse.mybir as mybir
    from concourse.tile import TileContext
    from concourse.masks import make_identity

    f32 = mybir.dt.float32
    bf16 = mybir.dt.bfloat16
    fp8 = mybir.dt.float8e4
    P = _P
    M_TILES = NB // P
    K_TILES = D // P
    K2 = K_TILES // 2  # DoubleRow k-pair count
    N = NB * ncores
    NCHUNK = N // CHUNK
    CP = CHUNK // P  # m-tiles per chunk width
    MG = 4  # m-tiles per PSUM group (4 banks)
    S2 = _SCALE * _SCALE
    assert NB % CHUNK == 0 and CHUNK % P == 0

    nc = bacc.Bacc(
        "TRN2", target_bir_lowering=False, debug=False, num_devices=ncores
    )
    xs = nc.dram_tensor("xs", [NB, D], f32, kind="ExternalInput")

    out = nc.dram_tensor("out", [P, M_TILES], f32, kind="ExternalOutput")

    with TileContext(nc) as tc:
        with (
            tc.tile_pool(name="consts", bufs=1) as consts,
            tc.tile_pool(name="dram", bufs=1, space="DRAM") as dram,
            tc.tile_pool(name="small", bufs=4) as small,
        ):
            identity = consts.tile([P, P], bf16)
            make_identity(nc, identity)
            pos16 = consts.tile([P, 2, P], fp8)
            nc.vector.memset(pos16, 0.0)
            nc.vector.tensor_scalar_mul(pos16[:, 0, :], identity, _SCALE)
            neg16 = consts.tile([P, 2, P], fp8)
            nc.vector.memset(neg16, 0.0)
            nc.vector.tensor_scalar_mul(neg16[:, 0, :], identity, -_SCALE)
            bias_dist = consts.tile([P, 1], f32)
            nc.vector.memset(bias_dist, 2.0 + _EPS_DIST)
            bias_log = consts.tile([P, 1], f32)
            nc.vector.memset(bias_log, _EPSILON)
            # resident transposed normalized shard (fp8, x16), split in
            # halves so the first AllGather's staging only depends on the
            # first half of the pre-pass (Tile deps are tile-granular)
            MH = M_TILES // 2
            xnT_A = consts.tile([P, K_TILES, MH * P], fp8)
            xnT_B = consts.tile([P, K_TILES, MH * P], fp8)
            maxacc = consts.tile([P, M_TILES, NCHUNK], f32)
            outt = consts.tile([P, M_TILES], f32)

            HB = NB // 2
            warm_in = dram.tile([P, 2], f32)
            warm_out = dram.tile([ncores * P, 2], f32, addr_space="Shared")
            xnT_localA = dram.tile([D, HB], fp8)
            xnT_localB = dram.tile([D, HB], fp8)
            xnT_allA = dram.tile([ncores * D, HB], fp8, addr_space="Shared")
            xnT_allB = dram.tile([ncores * D, HB], fp8, addr_space="Shared")

            # tiny warm-up AllGather issued first: it binds the launch
            # barrier and absorbs the first-collective setup cost during
            # the pre-pass window, so the real AllGathers start faster.
            wz = consts.tile([P, 2], f32)
            nc.vector.memset(wz, 0.0)
            nc.sync.dma_start(warm_in[:, :], wz)
            nc.gpsimd.collective_compute(
                "AllGather",
                mybir.AluOpType.bypass,
                replica_groups=[list(range(ncores))],
                ins=[warm_in.opt()],
                outs=[warm_out.opt()],
            )

            # ---- pre-pass: normalize own shard, transpose, stage for AG ----
            with (
                tc.tile_pool(name="prep", bufs=3) as prep,
                tc.tile_pool(name="ppsum", bufs=4, space="PSUM") as ppsum,
            ):
                for m in range(M_TILES):
                    xt = prep.tile([P, D], f32, tag="xt")
                    nc.sync.dma_start(xt, xs[m * P : (m + 1) * P, :])
                    sq = prep.tile([P, D], bf16, tag="sq")
                    ss = small.tile([P, 1], f32, tag="ss")
                    nc.scalar.activation(
                        sq,
                        xt,
                        mybir.ActivationFunctionType.Square,
                        accum_out=ss,
                    )
                    nrm = small.tile([P, 1], f32, tag="nrm")
                    nc.scalar.sqrt(nrm, ss)
                    nrm2 = small.tile([P, 1], f32, tag="nrm2")
                    nc.vector.tensor_scalar_max(nrm2, nrm, _EPS_NORM)
                    rinv = small.tile([P, 1], f32, tag="rinv")
                    nc.vector.reciprocal(rinv, nrm2)
                    rinv16 = small.tile([P, 1], f32, tag="rinv16")
                    nc.vector.tensor_scalar_mul(rinv16, rinv, _SCALE)
                    xnb = prep.tile([P, D], bf16, tag="xnb")
                    nc.scalar.mul(xnb, xt, rinv16)
                    for k in range(K_TILES):
                        pt = ppsum.tile([P, P], bf16, tag="pt")
                        nc.tensor.transpose(
                            pt, xnb[:, k * P : (k + 1) * P], identity
                        )
                        xnT_h = xnT_A if m < MH else xnT_B
                        nc.vector.tensor_copy(
                            xnT_h[:, k, (m % MH) * P : (m % MH + 1) * P], pt
                        )
                    # stage + gather each half as soon as its m-tiles are
                    # done so the first AllGather overlaps the second half of
                    # the pre-pass (and the launch-skew barrier).
                    if m == M_TILES // 2 - 1:
                        nc.sync.dma_start(
                            xnT_localA[:, :].rearrange(
                                "(k p) n -> p k n", p=P
                            ),
                            xnT_A,
                        )
                        nc.gpsimd.collective_compute(
                            "AllGather",
                            mybir.AluOpType.bypass,
                            replica_groups=[list(range(ncores))],
                            ins=[xnT_localA.opt()],
                            outs=[xnT_allA.opt()],
                        )
                    elif m == M_TILES - 1:
                        nc.sync.dma_start(
                            xnT_localB[:, :].rearrange(
                                "(k p) n -> p k n", p=P
                            ),
                            xnT_B,
                        )
                        nc.gpsimd.collective_compute(
                            "AllGather",
                            mybir.AluOpType.bypass,
                            replica_groups=[list(range(ncores))],
                            ins=[xnT_localB.opt()],
                            outs=[xnT_allB.opt()],
                        )

            # ---- main pass ----
            # Own-block chunks run straight out of SBUF with a compile-time
            # diagonal fix, overlapping the collective launch barrier; the
            # 14 remote chunks are fetched with a cc_rank-steered dynamic
            # DMA offset ((rank+1+i) & 7 source block), so the throttled
            # main loop does 14/16 of the work.
            from concourse.bass import ds

            with (
                tc.tile_pool(name="rhsp", bufs=14) as rhsp,
                tc.tile_pool(name="mpsum", bufs=2, space="PSUM") as mpsum,
            ):
                def gram_chunk(lhsAB, rhs_fn, slot, fix_h):
                    """One CHUNK-wide column chunk; rhs_fn(k2) -> [P,2,CHUNK]
                    moving AP. fix_h: chunk-half for the static diag fix or
                    None."""
                    for mg in range(M_TILES // MG):
                        psg = mpsum.tile([P, MG, CHUNK], f32, tag="ps")
                        for mi in range(MG):
                            m = mg * MG + mi
                            ps = psg[:, mi, :]
                            lhs_h = lhsAB[0] if m < MH else lhsAB[1]
                            lmc = (m % MH) * P
                            has_fix = fix_h is not None and m // CP == fix_h
                            for k2 in range(K2):
                                nc.tensor.matmul(
                                    ps,
                                    lhs_h[:, 2 * k2 : 2 * k2 + 2, lmc : lmc + P],
                                    rhs_fn(k2),
                                    start=(k2 == 0),
                                    stop=(k2 == K2 - 1) and not has_fix,
                                    perf_mode=mybir.MatmulPerfMode.DoubleRow,
                                )
                            if has_fix:
                                off = (m % CP) * P
                                nc.tensor.matmul(
                                    ps[:, off : off + P],
                                    pos16,
                                    neg16,
                                    start=False,
                                    stop=True,
                                    perf_mode=mybir.MatmulPerfMode.DoubleRow,
                                    skip_group_check=True,
                                )
                        nc.vector.reduce_max(
                            maxacc[:, mg * MG : (mg + 1) * MG, slot : slot + 1],
                            psg,
                            axis=mybir.AxisListType.X,
                        )

                # own block (slots 0,1): no AllGather dependency
                for h in range(2):
                    src_own = xnT_A if h == 0 else xnT_B
                    gram_chunk(
                        (xnT_A, xnT_B),
                        lambda k2, s=src_own: s[:, 2 * k2 : 2 * k2 + 2, :],
                        h,
                        h,
                    )

                # remote blocks (slots 2..15): rank-steered source offset
                rank = nc.sync.cc_rank(replica_groups=[list(range(ncores))])
                for half in range(2):
                    src = xnT_allA if half == 0 else xnT_allB
                    for i in range(ncores - 1):
                        blk = (rank + (1 + i)) & 7
                        rt = rhsp.tile([P, K_TILES, CHUNK], fp8, tag="rhs")
                        nc.sync.dma_start(
                            rt,
                            src[ds(blk * D, D), :].rearrange(
                                "(k p) c -> p k c", p=P
                            ),
                        )
                        slot = 2 + half * (ncores - 1) + i
                        gram_chunk(
                            (xnT_A, xnT_B),
                            lambda k2, r=rt: r[:, 2 * k2 : 2 * k2 + 2, :],
                            slot,
                            None,
                        )

            # ---- final: clamp, dist, log (vectorized over all m) ----
            mx = small.tile([P, M_TILES], f32, tag="mx")
            nc.vector.reduce_max(mx, maxacc, axis=mybir.AxisListType.X)
            mxc = small.tile([P, M_TILES], f32, tag="mxc")
            nc.vector.tensor_scalar_min(mxc, mx, S2)
            dst = small.tile([P, M_TILES], f32, tag="dst")
            nc.scalar.activation(
                dst,
                mxc,
                mybir.ActivationFunctionType.Sqrt,
                bias=bias_dist,
                scale=-2.0 / S2,
            )
            nc.scalar.activation(
                outt,
                dst,
                mybir.ActivationFunctionType.Ln,
                bias=bias_log,
                scale=1.0,
            )
            nc.sync.dma_start(out[:, :], outt)

    nc.compile()
    return nc


def _run(thought_vectors, trace=False, tmpdir=None):
    from concourse import mybir
    from concourse.bass_utils import run_bass_kernel_spmd


    ncores, NB, D, CHUNK = 8, 1024, 1024, 512
    x = np.ascontiguousarray(
        np.asarray(thought_vectors, dtype=np.float32).reshape(-1, D)
    )
    N = x.shape[0]
    assert N == ncores * NB

    nc = _build_program(ncores, NB, D, CHUNK)

    in_maps = []
    for c in range(ncores):
        in_maps.append({"xs": x[c * NB : (c + 1) * NB]})

    res = run_bass_kernel_spmd(
        nc,
        in_maps,
        core_ids=list(range(ncores)),
        trace=trace,
        tmpdir=tmpdir,
    )

    total = 0.0
    for c in range(ncores):
        total += float(np.asarray(res.results[c]["out"], dtype=np.float64).sum())
    loss = -total / N
    return np.float32(loss), res


def kernel(thought_vectors):
    loss, _ = _run(thought_vectors)
    return np.asarray(loss, dtype=np.float32)
